# revision 12
# baseline (speedup 1.0000x reference)
"""2D Haar DWT (single level) on Trainium2, 8 NeuronCores, pure data parallel.

Math: per-2x2-block butterflies (ll,lh,hl,hh) = 0.5*(x00 +/- x01 +/- x10
+/- x11).  bf16 crosses HBM both ways (host casts; *0.5 folded into the
PSUM evacuation): 4 MiB in + 4 MiB out per core.

Three engines share the work (measured rates, TRN2):
  DVE   width pass: pure contiguous [128,N]+/-[128,N] bf16 at 2x mode
        ((N/2+151)/0.96 ns) — host pre-deinterleaves even/odd columns
        so there is no strided access anywhere.
  PE    height pass: row parity lives on the PARTITION axis (host
        layout), so row-pair butterflies are one matmul with a +/-1
        stationary matrix B[128,128]: out[c*64+mm] = T[mm] +/- T[64+mm],
        accumulated exactly in f32 PSUM.
  ACT   (+ DVE for a few blocks) evacuates PSUM -> bf16 SBUF with the
        0.5 scale folded in ((N+352)/1.2 ns).

Input layout per core: [p=rp*64+mm][t][g][k][j], image row = 2*(64g+mm)
+rp, col = 2k+t.  T = [d][k][j] per partition.  PSUM partition c*64+mm.
Out DRAM is written in SBUF-native order (big contiguous runs); the
host un-permutes.  Units: (g0,k-half), (g0,k-half), g1, g2, g3 —
tapered so the first out-DMA starts early.  In-DMAs on the SP HWDGE
ring (2-4 KiB runs), out-DMAs on the ACT ring (4-8 KiB runs).
"""

import numpy as np
import ml_dtypes

import concourse.mybir as mybir
from concourse import bacc, tile
from concourse.bass_utils import run_bass_kernel_spmd

N_CORES = 8
BATCH = 64
B_PER = BATCH // N_CORES  # 8 images per core
H = W = 512

BF16 = ml_dtypes.bfloat16
# (g, k0, ks) column-slice units; free size per partition = 2*2*ks*8
UNITS = [(0, 0, 128), (0, 128, 128), (1, 0, 256), (2, 0, 256), (3, 0, 256)]
EVAC_DVE = {(3, 1), (4, 1)}  # (unit_idx, d) PSUM blocks evacuated by DVE

_nc_cache = None


def build_bass():
    bf16 = mybir.dt.bfloat16
    f32 = mybir.dt.float32
    nc = bacc.Bacc(
        "TRN2", target_bir_lowering=False, debug=False, num_devices=N_CORES
    )
    # [p][t][g][k][j]
    inp = nc.dram_tensor(
        "inputs", [128, 2, 4, 256, 8], bf16, kind="ExternalInput"
    ).ap()
    bmat = nc.dram_tensor("bmat", [128, 128], bf16, kind="ExternalInput").ap()
    # [p'][unit blocks of [d][kk][j]]
    out = nc.dram_tensor("out", [128, 16384], bf16, kind="ExternalOutput").ap()

    with tile.TileContext(nc) as tc:
        pool_cm = tc.tile_pool(name="p", bufs=3)
        pool = pool_cm.__enter__()
        ps_cm = tc.psum_pool(name="ps", bufs=2)
        psp = ps_cm.__enter__()

        lp_cm = nc.allow_low_precision(reason="bf16 DWT: rel-err budget 2e-2")
        lp_cm.__enter__()

        B = pool.tile([128, 128], bf16, tag="B", bufs=1)
        nc.sync.dma_start(out=B[:], in_=bmat[:])

        def unit(u, g, k0, ks, off):
            Q = ks * 8  # one (t) half of X / one (d) block of T
            F = 2 * Q
            X = pool.tile([128, F], bf16, tag="X", bufs=3)
            nc.sync.dma_start(
                out=X[:],
                in_=inp[:, :, g, k0 : k0 + ks, :],
            )
            # width pass on DVE: T = [d][kk][j]
            T = pool.tile([128, F], bf16, tag="T")
            nc.vector.tensor_add(
                out=T[:, 0:Q], in0=X[:, 0:Q], in1=X[:, Q : 2 * Q]
            )
            nc.vector.tensor_sub(
                out=T[:, Q : 2 * Q], in0=X[:, 0:Q], in1=X[:, Q : 2 * Q]
            )
            # height pass on PE + evac (0.5 scale + f32->bf16) on ACT/DVE
            Yb = pool.tile([128, F], bf16, tag="Yb", bufs=3)
            for d in range(2):
                ps = psp.tile([128, Q], f32, tag="ps")
                for c0 in range(0, Q, 512):
                    nc.tensor.matmul(
                        ps[:, c0 : c0 + 512],
                        B[:],
                        T[:, d * Q + c0 : d * Q + c0 + 512],
                    )
                dst = Yb[:, d * Q : (d + 1) * Q]
                if (u, d) in EVAC_DVE:
                    nc.vector.tensor_scalar_mul(dst, ps[:], 0.5)
                else:
                    nc.scalar.mul(dst, ps[:], 0.5)
            nc.scalar.dma_start(out=out[:, off : off + F], in_=Yb[:])

        off = 0
        for u, (g, k0, ks) in enumerate(UNITS):
            unit(u, g, k0, ks, off)
            off += 2 * ks * 8

        lp_cm.__exit__(None, None, None)
        ps_cm.__exit__(None, None, None)
        pool_cm.__exit__(None, None, None)

    nc.compile()
    return nc


def make_bmat():
    b = np.zeros((128, 128), dtype=np.float32)
    mm = np.arange(64)
    b[mm, mm] = 1.0  # c=0: +T[mm]
    b[64 + mm, mm] = 1.0  # c=0: +T[64+mm]
    b[mm, 64 + mm] = 1.0  # c=1: +T[mm]
    b[64 + mm, 64 + mm] = -1.0  # c=1: -T[64+mm]
    return b.astype(BF16)


def prep_inputs(x):
    """x: (64, 512, 512) f32 -> per-core [128, 2, 4, 256, 8] bf16."""
    # [B][g][mm][rp][k][t]: row = 128g + 2mm + rp = 2*(64g+mm)+rp, col = 2k+t
    arr = np.asarray(x, dtype=np.float32).reshape(BATCH, 4, 64, 2, 256, 2)
    arr = arr.astype(BF16)
    shards = []
    for c in range(N_CORES):
        blk = arr[c * B_PER : (c + 1) * B_PER]  # [j][g][mm][rp][k][t]
        blk = blk.transpose(3, 2, 5, 1, 4, 0)  # [rp][mm][t][g][k][j]
        shards.append(np.ascontiguousarray(blk).reshape(128, 2, 4, 256, 8))
    return shards


def assemble_output(outs):
    """outs: per-core [128, 16384] bf16 -> (64, 512, 512, 1) f32 (scaled)."""
    res = np.empty((BATCH, H, W), dtype=np.float32)
    for core, o in enumerate(outs):
        rc = res[core * B_PER : (core + 1) * B_PER]
        off = 0
        for g, k0, ks in UNITS:
            F = 2 * ks * 8
            blk = o[:, off : off + F].reshape(2, 64, 2, ks, 8)  # [c][mm][d][kk][j]
            for c in range(2):
                for d in range(2):
                    rc[
                        :,
                        c * 256 + 64 * g : c * 256 + 64 * g + 64,
                        d * 256 + k0 : d * 256 + k0 + ks,
                    ] = blk[c, :, d, :, :].transpose(2, 0, 1)
            off += F
    return res.reshape(BATCH, H, W, 1)


def kernel(**inputs):
    global _nc_cache
    x = np.asarray(inputs["inputs"], dtype=np.float32).reshape(BATCH, H, W)
    shards = prep_inputs(x)
    bm = make_bmat()
    if _nc_cache is None:
        _nc_cache = build_bass()
    nc = _nc_cache
    in_maps = [{"inputs": shards[i], "bmat": bm} for i in range(N_CORES)]
    res = run_bass_kernel_spmd(nc, in_maps, core_ids=list(range(N_CORES))).results
    return assemble_output([res[i]["out"] for i in range(N_CORES)])


# revision 14
# speedup vs baseline: 1.1595x; 1.1595x over previous
"""2D Haar DWT (single level) on Trainium2, 8 NeuronCores, pure data parallel.

Math: per-2x2-block butterflies (ll,lh,hl,hh) = 0.5*(x00 +/- x01 +/- x10
+/- x11).  bf16 crosses HBM both ways (host casts; *0.5 folded into the
PSUM evacuation): 4 MiB in + 4 MiB out per core.

The WHOLE transform is one matmul per 512-column chunk: the host puts
(row parity rp, column parity t) on the PARTITION axis —
p = rp*64 + t*32 + mm, image row = 2*m+rp with m = 32*g8+mm,
col = 2k+t — so each output element is
a +/-1 combination of 4 partitions with equal (mm): a single stationary
matrix B4[128,128] with B4[rp*64+t*32+mm, (2c+h)*32+mm] = s_c[rp]*s_h[t]
(s_0=[1,1], s_1=[1,-1]) computes ALL FOUR subbands at once:
PSUM[(2c+h)*32+mm][g8][k][j] = unscaled subband value, accumulated
exactly in f32.  PE (idle otherwise, 2.4 GHz) streams X through B4;
DVE and ACT split the PSUM -> bf16 SBUF evacuation with the 0.5 scale
folded in.  No width/height tensor ops at all, and only ONE bf16
rounding of the result (better precision than a two-pass butterfly).

Out DRAM is written in SBUF-native order (4-8 KiB runs); the host
un-permutes.  Units are g8-ranges [1,2,2,2,1] (0.5 MiB per g8, 4 KiB
in-runs) — tapered so the first out-DMA starts early.  In-DMAs on the
SP HWDGE ring, out-DMAs on the ACT ring.
"""

import numpy as np
import ml_dtypes

import concourse.mybir as mybir
from concourse import bacc, tile
from concourse.bass_utils import run_bass_kernel_spmd

N_CORES = 8
BATCH = 64
B_PER = BATCH // N_CORES  # 8 images per core
H = W = 512

BF16 = ml_dtypes.bfloat16
UNITS = [(0, 1), (1, 2), (3, 2), (5, 2), (7, 1)]  # (g8 start, n g8-blocks)

_nc_cache = None


def build_bass():
    bf16 = mybir.dt.bfloat16
    f32 = mybir.dt.float32
    nc = bacc.Bacc(
        "TRN2", target_bir_lowering=False, debug=False, num_devices=N_CORES
    )
    # [p = rp*64+t*32+mm][g8][k][j]
    inp = nc.dram_tensor(
        "inputs", [128, 8, 256, 8], bf16, kind="ExternalInput"
    ).ap()
    bmat = nc.dram_tensor("bmat", [128, 128], bf16, kind="ExternalInput").ap()
    # [p' = (2c+h)*32+mm][g8][k][j]
    out = nc.dram_tensor("out", [128, 16384], bf16, kind="ExternalOutput").ap()

    with tile.TileContext(nc) as tc:
        pool_cm = tc.tile_pool(name="p", bufs=3)
        pool = pool_cm.__enter__()
        ps_cm = tc.psum_pool(name="ps", bufs=4)
        psp = ps_cm.__enter__()

        lp_cm = nc.allow_low_precision(reason="bf16 DWT: rel-err budget 2e-2")
        lp_cm.__enter__()

        B = pool.tile([128, 128], bf16, tag="B", bufs=1)
        nc.sync.dma_start(out=B[:], in_=bmat[:])

        chunk_idx = [0]

        def unit(g0, ng, off):
            F = 2048 * ng
            X = pool.tile([128, F], bf16, tag="X", bufs=3)
            nc.sync.dma_start(out=X[:], in_=inp[:, g0 : g0 + ng, :, :])
            Yb = pool.tile([128, F], bf16, tag="Yb", bufs=3)
            for c0 in range(0, F, 1024):
                ps = psp.tile([128, 1024], f32, tag="ps")
                nc.tensor.matmul(ps[:, 0:512], B[:], X[:, c0 : c0 + 512])
                nc.tensor.matmul(ps[:, 512:1024], B[:], X[:, c0 + 512 : c0 + 1024])
                dst = Yb[:, c0 : c0 + 1024]
                if chunk_idx[0] % 2 == 0:
                    nc.vector.tensor_scalar_mul(dst, ps[:], 0.5)
                else:
                    nc.scalar.mul(dst, ps[:], 0.5)
                chunk_idx[0] += 1
            nc.scalar.dma_start(out=out[:, off : off + F], in_=Yb[:])

        off = 0
        for g0, ng in UNITS:
            unit(g0, ng, off)
            off += 2048 * ng

        lp_cm.__exit__(None, None, None)
        ps_cm.__exit__(None, None, None)
        pool_cm.__exit__(None, None, None)

    nc.compile()
    return nc


def make_bmat():
    b = np.zeros((128, 128), dtype=np.float32)
    mm = np.arange(32)
    sgn = [np.array([1.0, 1.0]), np.array([1.0, -1.0])]
    for rp in range(2):
        for t in range(2):
            for c in range(2):
                for h in range(2):
                    b[rp * 64 + t * 32 + mm, (2 * c + h) * 32 + mm] = (
                        sgn[c][rp] * sgn[h][t]
                    )
    return b.astype(BF16)


def prep_inputs(x):
    """x: (64, 512, 512) f32 -> per-core [128, 8, 256, 8] bf16."""
    # [B][g8][mm][rp][k][t]: row = 2*(32*g8+mm)+rp, col = 2k+t
    arr = np.asarray(x, dtype=np.float32).reshape(BATCH, 8, 32, 2, 256, 2)
    arr = arr.astype(BF16)
    shards = []
    for c in range(N_CORES):
        blk = arr[c * B_PER : (c + 1) * B_PER]  # [j][g8][mm][rp][k][t]
        blk = blk.transpose(3, 5, 2, 1, 4, 0)  # [rp][t][mm][g8][k][j]
        shards.append(np.ascontiguousarray(blk).reshape(128, 8, 256, 8))
    return shards


def assemble_output(outs):
    """outs: per-core [128, 16384] bf16 -> (64, 512, 512, 1) f32 (scaled)."""
    res = np.empty((BATCH, H, W), dtype=np.float32)
    for core, o in enumerate(outs):
        # [c][h][mm][g8][k][j] -> [j][c][g8][mm][h][k]
        blk = o.reshape(2, 2, 32, 8, 256, 8).transpose(5, 0, 3, 2, 1, 4)
        res[core * B_PER : (core + 1) * B_PER] = blk.reshape(B_PER, H, W)
    return res.reshape(BATCH, H, W, 1)


def kernel(**inputs):
    global _nc_cache
    x = np.asarray(inputs["inputs"], dtype=np.float32).reshape(BATCH, H, W)
    shards = prep_inputs(x)
    bm = make_bmat()
    if _nc_cache is None:
        _nc_cache = build_bass()
    nc = _nc_cache
    in_maps = [{"inputs": shards[i], "bmat": bm} for i in range(N_CORES)]
    res = run_bass_kernel_spmd(nc, in_maps, core_ids=list(range(N_CORES))).results
    return assemble_output([res[i]["out"] for i in range(N_CORES)])
